# revision 6
# baseline (speedup 1.0000x reference)
"""LSTM decoder kernel for Trainium2, 8 NeuronCores.

Strategy: data-parallel over batch (32 rows/core, no collectives).
Per-core matmuls are batch-major with 4x column tiling (each 32-wide
col-tile computes a different 256-col window of the gate dim, writing
PSUM partitions 32j:32j+32). x_proj (constant across timesteps) is
injected via fp32r identity-stationary matmuls; the recurrent
h @ W_hh.T runs in bf16. Cell state c stays fp32.

Pipelining within a step: gates are computed in three matmul phases
  FG (bank0, N=512: f|g cols), I (bank1 lo, N=256), O (bank1 hi, N=256)
so sigmoid(f), tanh(g), c*=f, i*g all overlap later matmul phases; only
sigmoid(o) + c+=tmp + tanh(c) + h + transpose are exposed. tanh/h/
transpose are split into two 128-col halves so the next step's matmuls
(which consume t1 = transpose of h[:, 0:128] H-chunks {0,2,4,6}) start
before the second half (t2) is finished.

Layout (per core, batch b in [0,32), col-tile j in [0,4)):
  psum gates [128, 1024]: [32j+b, :] covers H-window [256j, 256j+256)
    cols 0:256 = f, 256:512 = g, 512:768 = i, 768:1024 = o
  c/h tiles [128,256]: [32j+b, f] = state[b, 256j+f]
  hT chunks k: (t1 if k even else t2)[:, 32*(k//2):+32]
"""
import numpy as np
import ml_dtypes

import concourse.bass as bass
import concourse.mybir as mybir
import concourse.tile as tile
from concourse import bacc
from concourse import bass_utils

B, H, O, T, NCORES = 256, 1024, 512, 128, 8
BL = B // NCORES          # 32 batch rows per core
BF16 = mybir.dt.bfloat16
F32 = mybir.dt.float32
F32R = mybir.dt.float32r

_CACHE = {}

# k iteration order: t1 chunks (even) first so next step can start on t1
KS = (0, 2, 4, 6, 1, 3, 5, 7)


def _emit_gate_phase(nc, gates_ps, w_sb, t1, t2, lo, hi, moff):
    """One accumulation phase of h @ W_hh.T: psum cols [lo:hi),
    moving cols [moff + (hi-lo)*j ...] of each W chunk."""
    n = hi - lo
    for k in KS:
        tt = t1 if k % 2 == 0 else t2
        stat = tt[:, 32 * (k // 2):32 * (k // 2) + 32]
        for j in range(4):
            nc.tensor.matmul(
                gates_ps[32 * j:32 * (j + 1), lo:hi],
                stat,
                w_sb[k][:, moff + 512 * j:moff + 512 * j + n],
                start=False,
                stop=(k == KS[-1] and j == 3),
                tile_position=(0, 32 * j),
                skip_group_check=True,
            )


def _emit_xp_mms(nc, gates_ps, eyeb, xp_hi, xp_lo):
    """x_proj injection as bf16 hi + bf16 lo (exact to ~4e-6 rel).

    start=True clears the full psum bank for the written partition rows,
    so each (bank, j) region gets exactly one start=True matmul (hi)
    followed by an accumulating one (lo)."""
    for xp_sb, st in ((xp_hi, True), (xp_lo, False)):
        for bank in range(2):
            for j in range(4):
                nc.tensor.matmul(
                    gates_ps[32 * j:32 * (j + 1), 512 * bank:512 * (bank + 1)],
                    eyeb[:, 32 * bank:32 * bank + 32],
                    xp_sb[:, 2048 * bank + 512 * j:2048 * bank + 512 * (j + 1)],
                    start=st, stop=False,
                    tile_position=(0, 32 * j), skip_group_check=True,
                )


def _emit_y_mms(nc, y_ps, wl_sb, t1, t2):
    for k in KS:
        tt = t1 if k % 2 == 0 else t2
        stat = tt[:, 32 * (k // 2):32 * (k // 2) + 32]
        for j in range(4):
            nc.tensor.matmul(
                y_ps[32 * j:32 * (j + 1), :],
                stat,
                wl_sb[:, 512 * k + 128 * j:512 * k + 128 * j + 128],
                start=(k == KS[0]),
                stop=(k == KS[-1] and j == 3),
                tile_position=(0, 32 * j),
                skip_group_check=True,
            )


def _build(steps=T, y_slots=None):
    """y_slots: if set, y output rotates through y_slots slots (timing
    builds use a small constant-size output so T-contrast isolates
    device compute)."""
    ny = y_slots or steps
    nc = bacc.Bacc("TRN2", target_bir_lowering=False, debug=False,
                   num_devices=NCORES)
    w_d = nc.dram_tensor("W", [128, 8 * 4096], BF16, kind="ExternalInput").ap()
    wl_d = nc.dram_tensor("Wl", [128, 4096], BF16, kind="ExternalInput").ap()
    xph_d = nc.dram_tensor("xph", [128, 4096], BF16, kind="ExternalInput").ap()
    xpl_d = nc.dram_tensor("xpl", [128, 4096], BF16, kind="ExternalInput").ap()
    eyeb_d = nc.dram_tensor("eyeb", [128, 128], BF16, kind="ExternalInput").ap()
    y_d = nc.dram_tensor("y", [ny, 128, 128], F32, kind="ExternalOutput").ap()

    ACT = mybir.ActivationFunctionType
    mult = mybir.AluOpType.mult
    addop = mybir.AluOpType.add

    with tile.TileContext(nc) as tc:
        with tc.tile_pool(name="stat", bufs=1) as statp, \
             tc.tile_pool(name="sb", bufs=2) as sb, \
             tc.tile_pool(name="ps", bufs=2, space="PSUM") as ps:
            w_sb = []
            for k in range(8):
                wk = statp.tile([128, 4096], BF16, tag=f"W{k}")
                nc.sync.dma_start(wk[:], w_d[:, 4096 * k:4096 * (k + 1)])
                w_sb.append(wk)
            wl_sb = statp.tile([128, 4096], BF16, tag="Wl")
            nc.sync.dma_start(wl_sb[:], wl_d)
            xp_hi = statp.tile([128, 4096], BF16, tag="xph")
            nc.sync.dma_start(xp_hi[:], xph_d)
            xp_lo = statp.tile([128, 4096], BF16, tag="xpl")
            nc.sync.dma_start(xp_lo[:], xpl_d)
            eyeb = statp.tile([128, 128], BF16, tag="eyeb")
            nc.sync.dma_start(eyeb[:], eyeb_d)
            c_sb = statp.tile([128, 256], F32, tag="c")
            nc.gpsimd.memset(c_sb[:], 0.0)

            t1_prev = t2_prev = None
            gates_cur = ps.tile([128, 1024], F32, tag="gates")
            _emit_xp_mms(nc, gates_cur, eyeb, xp_hi, xp_lo)

            for t in range(steps):
                # --- PE: gate matmul phases (h(t-1) contribution) ---
                if t > 0:
                    _emit_gate_phase(nc, gates_cur, w_sb, t1_prev, t2_prev,
                                     0, 512, 0)        # f|g
                    _emit_gate_phase(nc, gates_cur, w_sb, t1_prev, t2_prev,
                                     512, 768, 2048)   # i
                    _emit_gate_phase(nc, gates_cur, w_sb, t1_prev, t2_prev,
                                     768, 1024, 2304)  # o

                # --- ACT: gate activations as their psum regions complete ---
                sf = sb.tile([128, 256], F32, tag="sf")
                nc.scalar.activation(sf[:], gates_cur[:, 0:256], ACT.Sigmoid)
                gt = sb.tile([128, 256], F32, tag="gt")
                nc.scalar.activation(gt[:], gates_cur[:, 256:512], ACT.Tanh)
                si = sb.tile([128, 256], F32, tag="si")
                nc.scalar.activation(si[:], gates_cur[:, 512:768], ACT.Sigmoid)
                so = sb.tile([128, 256], F32, tag="so")
                nc.scalar.activation(so[:], gates_cur[:, 768:1024], ACT.Sigmoid)

                # --- PE fill work for the chain window ---
                if t > 0:
                    y_ps = ps.tile([128, 128], F32, tag="y")
                    _emit_y_mms(nc, y_ps, wl_sb, t1_prev, t2_prev)
                    y_sb = sb.tile([128, 128], F32, tag="ysb")
                    nc.vector.tensor_copy(y_sb[:], y_ps[:])
                    nc.sync.dma_start(y_d[(t - 1) % ny], y_sb[:])
                if t < steps - 1:
                    gates_next = ps.tile([128, 1024], F32, tag="gates")
                    _emit_xp_mms(nc, gates_next, eyeb, xp_hi, xp_lo)

                # --- DVE/ACT: cell update ---
                nc.vector.tensor_tensor(c_sb[:], sf[:], c_sb[:], mult)
                tmp = sb.tile([128, 256], F32, tag="tmp")
                nc.vector.tensor_tensor(tmp[:], si[:], gt[:], mult)
                nc.vector.tensor_tensor(c_sb[:], c_sb[:], tmp[:], addop)

                # tail in two 128-col halves to pipeline ACT/DVE/PE
                th = sb.tile([128, 256], F32, tag="th")
                h_sb = sb.tile([128, 256], BF16, tag="h")
                tp = ps.tile([128, 256], BF16, tag="tp")
                t1 = sb.tile([128, 128], BF16, tag="t1")
                t2 = sb.tile([128, 128], BF16, tag="t2")
                for half, out_t in ((0, t1), (1, t2)):
                    sl = slice(128 * half, 128 * (half + 1))
                    nc.scalar.activation(th[:, sl], c_sb[:, sl], ACT.Tanh)
                    nc.vector.tensor_tensor(h_sb[:, sl], so[:, sl],
                                            th[:, sl], mult)
                    nc.tensor.transpose(tp[:, sl], h_sb[:, sl], eyeb[:])
                    nc.vector.tensor_copy(out_t[:], tp[:, sl])

                t1_prev, t2_prev = t1, t2
                if t < steps - 1:
                    gates_cur = gates_next

            y_ps = ps.tile([128, 128], F32, tag="y")
            _emit_y_mms(nc, y_ps, wl_sb, t1_prev, t2_prev)
            y_sb = sb.tile([128, 128], F32, tag="ysb")
            nc.vector.tensor_copy(y_sb[:], y_ps[:])
            nc.sync.dma_start(y_d[(steps - 1) % ny], y_sb[:])

    nc.compile()
    return nc


def _colmap():
    """Map device gate-column w -> original gate column.

    Device layout: cols [512j, 512j+512) = f|g for H-window j,
    cols [2048+512j, 2048+512j+512) = i|o for H-window j.
    Torch gate order in W rows: i(0), f(1), g(2), o(3)."""
    m = np.empty(4096, np.int64)
    ar = np.arange(256)
    for j in range(4):
        m[512 * j:512 * j + 256] = 1 * 1024 + 256 * j + ar            # f
        m[512 * j + 256:512 * (j + 1)] = 2 * 1024 + 256 * j + ar      # g
        m[2048 + 512 * j:2048 + 512 * j + 256] = 0 * 1024 + 256 * j + ar   # i
        m[2048 + 512 * j + 256:2048 + 512 * (j + 1)] = 3 * 1024 + 256 * j + ar  # o
    return m


def _prep_inputs(C, W_ih, W_hh, b_ih, b_hh, W_lin):
    xp = np.asarray(C, np.float32) @ np.asarray(W_ih, np.float32).T
    xp = xp + np.asarray(b_ih, np.float32) + np.asarray(b_hh, np.float32)
    cm = _colmap()
    w_perm = np.asarray(W_hh, np.float32).T[:, cm]
    w_dev = np.ascontiguousarray(
        w_perm.reshape(8, 128, 4096)
        .transpose(1, 0, 2).reshape(128, 8 * 4096)).astype(ml_dtypes.bfloat16)
    wl_dev = np.ascontiguousarray(
        np.asarray(W_lin, np.float32).T.reshape(8, 128, 512)
        .transpose(1, 0, 2).reshape(128, 4096)).astype(ml_dtypes.bfloat16)
    eyeb = np.eye(128, dtype=ml_dtypes.bfloat16)
    in_maps = []
    for c in range(NCORES):
        xpb = xp[BL * c:BL * (c + 1)][:, cm]   # [32, 4096] in device col order
        xp_c = np.zeros((128, 4096), np.float32)
        xp_c[0:32, 0:2048] = xpb[:, 0:2048]          # f|g rows
        xp_c[32:64, 2048:4096] = xpb[:, 2048:4096]   # i|o rows
        xp_h = xp_c.astype(ml_dtypes.bfloat16)
        xp_l = (xp_c - xp_h.astype(np.float32)).astype(ml_dtypes.bfloat16)
        in_maps.append({"W": w_dev, "Wl": wl_dev, "xph": xp_h, "xpl": xp_l,
                        "eyeb": eyeb})
    return in_maps


def kernel(C, W_ih, W_hh, b_ih, b_hh, W_lin, b_lin, max_seq_len):
    assert int(max_seq_len) == T and C.shape == (B, H)
    if "nc" not in _CACHE:
        _CACHE["nc"] = _build()
    nc = _CACHE["nc"]
    in_maps = _prep_inputs(C, W_ih, W_hh, b_ih, b_hh, W_lin)
    try:
        res = bass_utils.run_bass_kernel_spmd(
            nc, in_maps, core_ids=list(range(NCORES)))
    except Exception:
        # transient NRT faults have been observed on this fabric; retry once
        res = bass_utils.run_bass_kernel_spmd(
            nc, in_maps, core_ids=list(range(NCORES)))
    out = np.empty((T, B, O), np.float32)
    blin = np.asarray(b_lin, np.float32)
    for c in range(NCORES):
        yc = res.results[c]["y"]          # [T, 128, 128]
        out[:, BL * c:BL * (c + 1), :] = (
            yc.reshape(T, 4, BL, 128).transpose(0, 2, 1, 3).reshape(T, BL, O)
            + blin)
    return out


# revision 12
# speedup vs baseline: 1.1743x; 1.1743x over previous
"""LSTM decoder kernel for Trainium2, 8 NeuronCores.

Strategy: data-parallel over batch (32 rows/core, no collectives).
Per-core matmuls are batch-major with 4x column tiling (each 32-wide
col-tile computes a different 256-col window of the gate dim, writing
PSUM partitions 32j:32j+32). x_proj (constant across timesteps) is
injected via bf16 hi+lo selector matmuls (exact to ~4e-6); the
recurrent h @ W_hh.T runs in bf16. Cell state c stays fp32.

Pipelining within a step: gates are computed in three matmul phases
  FG (bank0, N=512: f|g cols), I (bank1 lo, N=256), O (bank1 hi, N=256)
so sigmoid(f), tanh(g), c*=f, i*g all overlap later matmul phases; only
sigmoid(o) + c+=tmp + tanh(c) + h + transpose are exposed. tanh/h/
transpose are split into two 128-col halves so the next step's matmuls
(which consume t1 = transpose of h[:, 0:128] H-chunks {0,2,4,6}) start
before the second half (t2) is finished.

Layout (per core, batch b in [0,32), col-tile j in [0,4)):
  psum gates [128, 1024]: [32j+b, :] covers H-window [256j, 256j+256)
    cols 0:256 = f, 256:512 = g, 512:768 = i, 768:1024 = o
  c/h tiles [128,256]: [32j+b, f] = state[b, 256j+f]
  hT chunks k: (t1 if k even else t2)[:, 32*(k//2):+32]
"""
import numpy as np
import ml_dtypes

import concourse.bass as bass
import concourse.mybir as mybir
import concourse.tile as tile
from concourse import bacc
from concourse import bass_utils

B, H, O, T, NCORES = 256, 1024, 512, 128, 8
BL = B // NCORES          # 32 batch rows per core
BF16 = mybir.dt.bfloat16
F32 = mybir.dt.float32
F32R = mybir.dt.float32r

_CACHE = {}

# k iteration order: t1 chunks (even) first so next step can start on t1
KS = (0, 2, 4, 6, 1, 3, 5, 7)


def _emit_gate_phase(nc, gates_ps, w_sb, t1, t2, lo, hi, moff):
    """One accumulation phase of h @ W_hh.T: psum cols [lo:hi),
    moving cols [moff + (hi-lo)*j ...] of each W chunk."""
    n = hi - lo
    for k in KS:
        tt = t1 if k % 2 == 0 else t2
        stat = tt[:, 32 * (k // 2):32 * (k // 2) + 32]
        for j in range(4):
            nc.tensor.matmul(
                gates_ps[32 * j:32 * (j + 1), lo:hi],
                stat,
                w_sb[k][:, moff + 512 * j:moff + 512 * j + n],
                start=False,
                stop=(k == KS[-1] and j == 3),
                tile_position=(0, 32 * j),
                skip_group_check=True,
            )


def _emit_xp_mms(nc, gates_ps, selhl, xp_sb):
    """x_proj injection, exact to ~4e-6 rel: xp is stored as bf16 hi and
    bf16 lo halves in different partition bands of xp_sb, and the
    stationary selector has two 1s per column so one K=128 matmul sums
    hi + lo. One start=True matmul per (bank, j) region (start=True
    clears the full psum bank for the written partition rows)."""
    for bank in range(2):
        for j in range(4):
            nc.tensor.matmul(
                gates_ps[32 * j:32 * (j + 1), 512 * bank:512 * (bank + 1)],
                selhl[:, 32 * bank:32 * bank + 32],
                xp_sb[:, 2048 * bank + 512 * j:2048 * bank + 512 * (j + 1)],
                start=True, stop=False,
                tile_position=(0, 32 * j), skip_group_check=True,
            )


def _emit_y_mms(nc, y_ps, wl_sb, t1, t2):
    for k in KS:
        tt = t1 if k % 2 == 0 else t2
        stat = tt[:, 32 * (k // 2):32 * (k // 2) + 32]
        for j in range(4):
            nc.tensor.matmul(
                y_ps[32 * j:32 * (j + 1), :],
                stat,
                wl_sb[:, 512 * k + 128 * j:512 * k + 128 * j + 128],
                start=(k == KS[0]),
                stop=(k == KS[-1] and j == 3),
                tile_position=(0, 32 * j),
                skip_group_check=True,
            )


def _build(steps=T, y_slots=None, sever=False):
    """y_slots: if set, y output rotates through y_slots slots (timing
    builds use a small constant-size output so T-contrast isolates
    device compute)."""
    ny = y_slots or steps
    nc = bacc.Bacc("TRN2", target_bir_lowering=False, debug=False,
                   num_devices=NCORES)
    w_d = nc.dram_tensor("W", [128, 8 * 4096], BF16, kind="ExternalInput").ap()
    wl_d = nc.dram_tensor("Wl", [128, 4096], BF16, kind="ExternalInput").ap()
    xpc_d = nc.dram_tensor("xpc", [128, 4096], BF16, kind="ExternalInput").ap()
    selhl_d = nc.dram_tensor("selhl", [128, 64], BF16, kind="ExternalInput").ap()
    eyeb_d = nc.dram_tensor("eyeb", [128, 128], BF16, kind="ExternalInput").ap()
    y_d = nc.dram_tensor("y", [ny, 128, 128], F32, kind="ExternalOutput").ap()

    ACT = mybir.ActivationFunctionType
    mult = mybir.AluOpType.mult
    addop = mybir.AluOpType.add

    with tile.TileContext(nc) as tc:
        with tc.tile_pool(name="stat", bufs=1) as statp, \
             tc.tile_pool(name="sb", bufs=2) as sb, \
             tc.tile_pool(name="ps", bufs=2, space="PSUM") as ps:
            w_sb = []
            for k in range(8):
                wk = statp.tile([128, 4096], BF16, tag=f"W{k}")
                nc.sync.dma_start(wk[:], w_d[:, 4096 * k:4096 * (k + 1)])
                w_sb.append(wk)
            wl_sb = statp.tile([128, 4096], BF16, tag="Wl")
            nc.sync.dma_start(wl_sb[:], wl_d)
            xpc = statp.tile([128, 4096], BF16, tag="xpc")
            nc.sync.dma_start(xpc[:], xpc_d)
            selhl = statp.tile([128, 64], BF16, tag="selhl")
            nc.sync.dma_start(selhl[:], selhl_d)
            eyeb = statp.tile([128, 128], BF16, tag="eyeb")
            nc.sync.dma_start(eyeb[:], eyeb_d)
            c_sb = statp.tile([128, 256], F32, tag="c")
            nc.gpsimd.memset(c_sb[:], 0.0)
            t1s = t2s = None
            if sever:
                t1s = statp.tile([128, 128], BF16, tag="t1s")
                nc.gpsimd.memset(t1s[:], 0.01)
                t2s = statp.tile([128, 128], BF16, tag="t2s")
                nc.gpsimd.memset(t2s[:], 0.01)

            t1_prev = t2_prev = None
            gates_cur = ps.tile([128, 1024], F32, tag="gates")
            _emit_xp_mms(nc, gates_cur, selhl, xpc)

            for t in range(steps):
                # --- PE: gate matmul phases (h(t-1) contribution) ---
                if t > 0:
                    g1, g2 = (t1s, t2s) if sever else (t1_prev, t2_prev)
                    _emit_gate_phase(nc, gates_cur, w_sb, g1, g2,
                                     0, 512, 0)        # f|g
                    _emit_gate_phase(nc, gates_cur, w_sb, g1, g2,
                                     512, 768, 2048)   # i
                    _emit_gate_phase(nc, gates_cur, w_sb, g1, g2,
                                     768, 1024, 2304)  # o

                # --- ACT: gate activations as their psum regions complete ---
                sf = sb.tile([128, 256], F32, tag="sf")
                nc.scalar.activation(sf[:], gates_cur[:, 0:256], ACT.Sigmoid)
                gt = sb.tile([128, 256], F32, tag="gt")
                nc.scalar.activation(gt[:], gates_cur[:, 256:512], ACT.Tanh)
                si = sb.tile([128, 256], F32, tag="si")
                nc.scalar.activation(si[:], gates_cur[:, 512:768], ACT.Sigmoid)
                so = sb.tile([128, 256], F32, tag="so")
                nc.scalar.activation(so[:], gates_cur[:, 768:1024], ACT.Sigmoid)

                # --- PE fill work for the chain window ---
                y_ps = None
                if t > 0:
                    g1, g2 = (t1s, t2s) if sever else (t1_prev, t2_prev)
                    y_ps = ps.tile([128, 128], F32, tag="y")
                    _emit_y_mms(nc, y_ps, wl_sb, g1, g2)
                if t < steps - 1:
                    gates_next = ps.tile([128, 1024], F32, tag="gates")
                    _emit_xp_mms(nc, gates_next, selhl, xpc)

                # --- DVE/ACT: cell update ---
                nc.vector.tensor_tensor(c_sb[:], sf[:], c_sb[:], mult)
                tmp = sb.tile([128, 256], F32, tag="tmp")
                nc.vector.tensor_tensor(tmp[:], si[:], gt[:], mult)
                nc.vector.tensor_tensor(c_sb[:], c_sb[:], tmp[:], addop)

                # tail in two 128-col halves to pipeline ACT/DVE/PE
                th = sb.tile([128, 256], F32, tag="th")
                h_sb = sb.tile([128, 256], BF16, tag="h")
                tp = ps.tile([128, 256], BF16, tag="tp")
                t1 = sb.tile([128, 128], BF16, tag="t1")
                t2 = sb.tile([128, 128], BF16, tag="t2")
                for half, out_t in ((0, t1), (1, t2)):
                    sl = slice(128 * half, 128 * (half + 1))
                    nc.scalar.activation(th[:, sl], c_sb[:, sl], ACT.Tanh)
                    nc.vector.tensor_tensor(h_sb[:, sl], so[:, sl],
                                            th[:, sl], mult)
                    nc.tensor.transpose(tp[:, sl], h_sb[:, sl], eyeb[:])
                    nc.vector.tensor_copy(out_t[:], tp[:, sl])

                # y copy emitted after the tail so the scheduler keeps the
                # DVE cell-update chain ahead of it in the FIFO
                if y_ps is not None:
                    y_sb = sb.tile([128, 128], F32, tag="ysb")
                    nc.vector.tensor_copy(y_sb[:], y_ps[:])
                    nc.sync.dma_start(y_d[(t - 1) % ny], y_sb[:])

                t1_prev, t2_prev = t1, t2
                if t < steps - 1:
                    gates_cur = gates_next

            y_ps = ps.tile([128, 128], F32, tag="y")
            _emit_y_mms(nc, y_ps, wl_sb, t1_prev, t2_prev)
            y_sb = sb.tile([128, 128], F32, tag="ysb")
            nc.vector.tensor_copy(y_sb[:], y_ps[:])
            nc.sync.dma_start(y_d[(steps - 1) % ny], y_sb[:])

    nc.compile()
    return nc


def _colmap():
    """Map device gate-column w -> original gate column.

    Device layout: cols [512j, 512j+512) = f|g for H-window j,
    cols [2048+512j, 2048+512j+512) = i|o for H-window j.
    Torch gate order in W rows: i(0), f(1), g(2), o(3)."""
    m = np.empty(4096, np.int64)
    ar = np.arange(256)
    for j in range(4):
        m[512 * j:512 * j + 256] = 1 * 1024 + 256 * j + ar            # f
        m[512 * j + 256:512 * (j + 1)] = 2 * 1024 + 256 * j + ar      # g
        m[2048 + 512 * j:2048 + 512 * j + 256] = 0 * 1024 + 256 * j + ar   # i
        m[2048 + 512 * j + 256:2048 + 512 * (j + 1)] = 3 * 1024 + 256 * j + ar  # o
    return m


def _prep_inputs(C, W_ih, W_hh, b_ih, b_hh, W_lin):
    xp = np.asarray(C, np.float32) @ np.asarray(W_ih, np.float32).T
    xp = xp + np.asarray(b_ih, np.float32) + np.asarray(b_hh, np.float32)
    cm = _colmap()
    w_perm = np.asarray(W_hh, np.float32).T[:, cm]
    w_dev = np.ascontiguousarray(
        w_perm.reshape(8, 128, 4096)
        .transpose(1, 0, 2).reshape(128, 8 * 4096)).astype(ml_dtypes.bfloat16)
    wl_dev = np.ascontiguousarray(
        np.asarray(W_lin, np.float32).T.reshape(8, 128, 512)
        .transpose(1, 0, 2).reshape(128, 4096)).astype(ml_dtypes.bfloat16)
    eyeb = np.eye(128, dtype=ml_dtypes.bfloat16)
    selhl = np.zeros((128, 64), ml_dtypes.bfloat16)
    for m in range(32):
        selhl[m, m] = 1            # fg selector: hi band
        selhl[32 + m, m] = 1       # fg selector: lo band
        selhl[64 + m, 32 + m] = 1  # io selector: hi band
        selhl[96 + m, 32 + m] = 1  # io selector: lo band
    in_maps = []
    for c in range(NCORES):
        xpb = xp[BL * c:BL * (c + 1)][:, cm]   # [32, 4096] in device col order
        hi = xpb.astype(ml_dtypes.bfloat16)
        lo = (xpb - hi.astype(np.float32)).astype(ml_dtypes.bfloat16)
        xpc = np.zeros((128, 4096), ml_dtypes.bfloat16)
        xpc[0:32, 0:2048] = hi[:, 0:2048]            # f|g hi band
        xpc[32:64, 0:2048] = lo[:, 0:2048]           # f|g lo band
        xpc[64:96, 2048:4096] = hi[:, 2048:4096]     # i|o hi band
        xpc[96:128, 2048:4096] = lo[:, 2048:4096]    # i|o lo band
        in_maps.append({"W": w_dev, "Wl": wl_dev, "xpc": xpc,
                        "selhl": selhl, "eyeb": eyeb})
    return in_maps


def kernel(C, W_ih, W_hh, b_ih, b_hh, W_lin, b_lin, max_seq_len):
    assert int(max_seq_len) == T and C.shape == (B, H)
    if "nc" not in _CACHE:
        _CACHE["nc"] = _build()
    nc = _CACHE["nc"]
    in_maps = _prep_inputs(C, W_ih, W_hh, b_ih, b_hh, W_lin)
    try:
        res = bass_utils.run_bass_kernel_spmd(
            nc, in_maps, core_ids=list(range(NCORES)))
    except Exception:
        # transient NRT faults have been observed on this fabric; retry once
        res = bass_utils.run_bass_kernel_spmd(
            nc, in_maps, core_ids=list(range(NCORES)))
    out = np.empty((T, B, O), np.float32)
    blin = np.asarray(b_lin, np.float32)
    for c in range(NCORES):
        yc = res.results[c]["y"]          # [T, 128, 128]
        out[:, BL * c:BL * (c + 1), :] = (
            yc.reshape(T, 4, BL, 128).transpose(0, 2, 1, 3).reshape(T, BL, O)
            + blin)
    return out
